# revision 11
# baseline (speedup 1.0000x reference)
"""Trainium2 Bass kernel for nn_BezierParameterProcessor (raw Bass).

Data-parallel over batch: B=16 -> 2 batches per core on 8 cores.

The KDE gaussian over the [-1,1]^2 tensor-product grid is separable:
    exp(-((gx-px)^2+(gy-py)^2)/(2 bw^2)) = Ex[n,w] * Ey[n,h]
so density/field reduce to per-h-chunk matmuls contracting n:
    dens[h,w]    = sum_n Ey[n,h] *  Ex[n,w]
    field_c[h,w] = sum_n Ey[n,h] * (Ex[n,w] * c0*valid[n]*vecs[n,c])
This needs 2*N*256 exps per batch instead of N*65536, and the whole
[B,HW,N] intermediate never exists. sigmoid(z) = 0.5*tanh(z/2)+0.5 keeps
every ACT call in the single `exp_and_others` table set (exp, tanh,
square all live there -> one ACT_TABLE_LOAD).

fp16 is used for matmul operands (fp32 PSUM accumulation): the gaussian
factors live in [0,1] and the MLP activations are O(1), so fp16 costs
~5e-4 relative error while running the PE single-pass (fp32 matmuls
decompose into 2x LDWEIGHTS + 2x MATMUL at ~4x the cost).

Raw Bass (no Tile framework): hand-placed engine programs with explicit
semaphores.  The Tile scheduler's fixed preamble/epilogue (semaphore
init storm + kernel-tail drain/barriers) costs ~15us on this toolchain,
more than the whole computation.

Inputs are packed into two [128, F] blobs (one per dtype): 2 input DMAs.
Point coords are stored NEGATED so (g - p)^2 is one ACT Square with the
coord as per-partition bias.  All six [128,256] output maps of a batch
are built in one [128,1536] SBUF tile and shipped with a single DMA to a
partition-major scratch layout; the host untangles it while unsharding.

PSUM bank map: A(1) = mlp1/mlp3 + vecs, B(1) = mlp2, R0(3), R1(3).
Per-batch psum R holds dens|f0|f1 in separate banks, so ACT can read the
dens bank while the PE still writes the field banks.
"""

import math
from contextlib import ExitStack

import numpy as np

import concourse.bass as bass
from concourse import mybir
from concourse.bass_utils import run_bass_kernel_spmd

H = W = 256
HID = 128
B = 16
N = 128  # points per batch (C*P = 16*8)
NCORES = 8
BS = B // NCORES  # batches per core = 2

FP32 = mybir.dt.float32
FP16 = mybir.dt.float16

# blob32 column layout: gx | gy | npc (negated coords) | bf | b3
C_GX, C_GY, C_PC, C_BF, C_B3 = 0, 256, 512, 516, 518
NC32 = 519
# blob16 column layout: w1t | w2t | w3t | wft | x0
C_W1, C_W2, C_W3, C_WF, C_X0 = 0, 64, 192, 320, 322
NC16 = 578

LAST_RESULT = None  # BassKernelResults of the most recent run (for profiling)


def _build(neg_inv2bw2, c0, sig_half_scale, sig_half_bias):
    AL = mybir.AluOpType
    ACT = mybir.ActivationFunctionType
    nc = bass.Bass("TRN2", target_bir_lowering=False)

    b32_d = nc.declare_dram_parameter("b32", [128, NC32], FP32, isOutput=False)
    b16_d = nc.declare_dram_parameter("b16", [128, NC16], FP16, isOutput=False)
    # scratch layout: [b][partition][1536] = dens(2x256) | f0(2x256) | f1(2x256)
    out_d = nc.declare_dram_parameter("out_o", [BS, 128, 1536], FP32, isOutput=True)

    ctx = ExitStack()
    with ctx:
        sb = lambda nm, shape, dt: ctx.enter_context(nc.sbuf_tensor(nm, shape, dt))
        ps = lambda nm, shape: ctx.enter_context(nc.psum_tensor(nm, shape, FP32))
        sem = lambda name: ctx.enter_context(nc.semaphore(name))

        b32 = sb("b32s", [128, NC32], FP32)
        b16 = sb("b16s", [128, NC16], FP16)
        sigb = sb("sigb", [128, 1], FP32)
        h1 = sb("h1", [65, BS * N], FP16)
        h2 = sb("h2", [HID, BS * N], FP16)
        enc = sb("enc", [HID, BS * N], FP16)
        vb = [sb(f"vb{b}", [N, 2], FP32) for b in range(BS)]
        vc = [sb(f"vcv{b}", [N, 2], FP32) for b in range(BS)]
        ub = [sb(f"ub{b}", [N, 2], FP32) for b in range(BS)]
        txs = sb("txs", [128, BS * W], FP32)
        tys = sb("tys", [128, BS * H], FP32)
        ey = sb("ey", [128, BS * H], FP16)
        exa = [sb(f"exa{b}", [128, 2 * W], FP16) for b in range(BS)]
        exu1 = [sb(f"exu1_{b}", [128, W], FP16) for b in range(BS)]
        obuf = [sb(f"obuf{b}", [128, 1536], FP32) for b in range(BS)]

        psA = ps("psA", [128, BS * N])  # mlp 1+3 outs, then vecs at cols 0:2 / 4:6
        psB = ps("psB", [128, BS * N])  # mlp layer 2 out
        psR = [ps(f"psR{b}", [128, 1536]) for b in range(BS)]  # dens|f0|f1

        sB32, sB16 = sem("sB32"), sem("sB16")
        sPE, sACT, sDVE = sem("sPE"), sem("sACT"), sem("sDVE")
        sGP, sOUT = sem("sGP"), sem("sOUT")

        gx = b32[:, C_GX : C_GX + W]
        gy = b32[:, C_GY : C_GY + H]
        bf = b32[:, C_BF : C_BF + 2]
        b3c = b32[:, C_B3 : C_B3 + 1]
        npc = lambda b, c: b32[:, C_PC + 2 * b + c : C_PC + 2 * b + c + 1]
        w1 = b16[:3, C_W1 : C_W1 + 64]
        w2 = b16[:65, C_W2 : C_W2 + HID]
        w3 = b16[:, C_W3 : C_W3 + HID]
        wf = b16[:, C_WF : C_WF + 2]
        x0 = b16[:3, C_X0 : C_X0 + BS * N]

        with nc.Block() as block:

            @block.gpsimd
            def _(gp):
                gp.memset(h1[64:65, :], 1.0).then_inc(sGP)  # GP1: layer-2 bias row
                gp.memset(sigb[:], sig_half_bias).then_inc(sGP)  # GP2

            @block.scalar
            def _(act):
                act.dma_start(out=b32[:], in_=b32_d[:]).then_inc(sB32, 16)
                act.wait_ge(sB32, 16)
                # (gy - py)^2 via Square(gy + npy): A1, A2
                for b in range(BS):
                    nc.scalar.activation(
                        tys[:, b * H : (b + 1) * H], gy, ACT.Square, bias=npc(b, 1)
                    ).then_inc(sACT)
                act.wait_ge(sACT, 2)  # ACT pipeline: tys written before read
                nc.scalar.activation(
                    ey[:], tys[:], ACT.Exp, scale=neg_inv2bw2
                ).then_inc(sACT)  # A3
                act.wait_ge(sDVE, 4)  # txs squared
                for b in range(BS):  # A4, A5
                    nc.scalar.activation(
                        exa[b][:, 0:W],
                        txs[:, b * W : (b + 1) * W],
                        ACT.Exp,
                        scale=neg_inv2bw2,
                    ).then_inc(sACT)
                # epilogue: sigmoid(s*x - thr) = 0.5*tanh(s/2*x - thr/2) + 0.5
                act.wait_ge(sGP, 2)
                for b in range(BS):  # A6..A9
                    act.wait_ge(sPE, 7 + 6 * b)  # dens matmuls of batch b done
                    nc.scalar.activation(
                        obuf[b][:, 0:512],
                        psR[b][:, 0:512],
                        ACT.Tanh,
                        scale=sig_half_scale,
                        bias=sigb[:],
                    ).then_inc(sACT)
                    act.wait_ge(sPE, 11 + 6 * b)  # field matmuls of batch b done
                    nc.scalar.activation(
                        obuf[b][:, 512:1536], psR[b][:, 512:1536], ACT.Tanh
                    ).then_inc(sACT)

            @block.vector
            def _(dve):
                dve.wait_ge(sPE, 1)
                nc.vector.tensor_scalar(
                    h1[:64, :], psA[:64, :], 0.0, None, AL.max
                ).then_inc(sDVE)  # D1: relu1
                dve.wait_ge(sB32, 16)
                for b in range(BS):  # D2, D3: gx + npx
                    nc.vector.tensor_scalar(
                        txs[:, b * W : (b + 1) * W], gx, npc(b, 0), None, AL.add
                    ).then_inc(sDVE)
                dve.wait_ge(sDVE, 3)  # txs subs landed (same-engine RAW)
                nc.vector.tensor_tensor(txs[:], txs[:], txs[:], AL.mult).then_inc(
                    sDVE
                )  # D4
                dve.wait_ge(sPE, 2)
                nc.vector.tensor_scalar(h2[:], psB[:], 0.0, None, AL.max).then_inc(
                    sDVE
                )  # D5: relu2
                dve.wait_ge(sPE, 3)
                nc.vector.tensor_scalar(enc[:], psA[:], b3c, None, AL.add).then_inc(
                    sDVE
                )  # D6: + b3
                for b in range(BS):  # D7..D12 / D13..D18
                    dve.wait_ge(sPE, 4 + b)
                    nc.vector.tensor_tensor(
                        vb[b][:], psA[:N, 4 * b : 4 * b + 2], bf, AL.add
                    ).then_inc(sDVE)
                    npx, npy = npc(b, 0), npc(b, 1)
                    nc.vector.tensor_scalar(
                        vc[b][:, 0:1], npx, -1.0, npx, AL.mult, AL.max
                    ).then_inc(sDVE)
                    nc.vector.tensor_scalar(
                        vc[b][:, 1:2], npy, -1.0, npy, AL.mult, AL.max
                    ).then_inc(sDVE)
                    dve.wait_ge(sDVE, 9 + 6 * b)
                    nc.vector.tensor_tensor(
                        vc[b][:, 0:1], vc[b][:, 0:1], vc[b][:, 1:2], AL.max
                    ).then_inc(sDVE)
                    dve.wait_ge(sDVE, 10 + 6 * b)
                    nc.vector.tensor_scalar(
                        vc[b][:, 0:1], vc[b][:, 0:1], 1e-8, c0, AL.is_gt, AL.mult
                    ).then_inc(sDVE)
                    dve.wait_ge(sDVE, 11 + 6 * b)
                    nc.vector.tensor_scalar(
                        ub[b][:], vb[b][:], vc[b][:, 0:1], None, AL.mult
                    ).then_inc(sDVE)
                for b in range(BS):  # D19/20, D21/22
                    dve.wait_ge(sDVE, 12 + 6 * b)  # ub[b] landed
                    dve.wait_ge(sACT, 4 + b)
                    nc.vector.tensor_scalar(
                        exa[b][:, W : 2 * W],
                        exa[b][:, 0:W],
                        ub[b][:, 0:1],
                        None,
                        AL.mult,
                    ).then_inc(sDVE)
                    nc.vector.tensor_scalar(
                        exu1[b][:], exa[b][:, 0:W], ub[b][:, 1:2], None, AL.mult
                    ).then_inc(sDVE)
                for b in range(BS):  # D23, D24
                    dve.wait_ge(sACT, 6 + 2 * b)
                    nc.vector.tensor_scalar(
                        obuf[b][:, 0:512], obuf[b][:, 0:512], 0.5, 0.5, AL.mult, AL.add
                    ).then_inc(sDVE)

            @block.tensor
            def _(pe):
                pe.wait_ge(sB16, 16)
                nc.tensor.matmul(psA[:64, :], w1, x0).then_inc(sPE)  # P1
                pe.wait_ge(sDVE, 1)
                pe.wait_ge(sGP, 1)
                nc.tensor.matmul(psB[:], w2, h1[:]).then_inc(sPE)  # P2
                pe.wait_ge(sDVE, 5)
                nc.tensor.matmul(psA[:], w3, h2[:]).then_inc(sPE)  # P3
                for b in range(BS):  # P4, P5: vecs
                    # psA bank0 is read by DVE (encAdd, vb[b-1]) — wait for the
                    # read to finish before writing the same bank (P10 hazard)
                    pe.wait_ge(sDVE, 6 + b)
                    nc.tensor.matmul(
                        psA[:N, 4 * b : 4 * b + 2], enc[:, b * N : (b + 1) * N], wf
                    ).then_inc(sPE)
                for b in range(BS):
                    pe.wait_ge(sACT, 4 + b)  # exa[b] Ex half (implies ey)
                    for ch in range(2):  # P6,P7 / P12,P13: dens
                        lhs = ey[:, b * H + ch * 128 : b * H + (ch + 1) * 128]
                        nc.tensor.matmul(
                            psR[b][:, ch * W : (ch + 1) * W], lhs, exa[b][:, 0:W]
                        ).then_inc(sPE)
                    pe.wait_ge(sDVE, 20 + 2 * b)  # exa[b] U-half + exu1[b]
                    for ch in range(2):  # P8..P11 / P14..P17: fields
                        lhs = ey[:, b * H + ch * 128 : b * H + (ch + 1) * 128]
                        nc.tensor.matmul(
                            psR[b][:, 512 + ch * W : 512 + (ch + 1) * W],
                            lhs,
                            exa[b][:, W : 2 * W],
                        ).then_inc(sPE)
                        nc.tensor.matmul(
                            psR[b][:, 1024 + ch * W : 1024 + (ch + 1) * W],
                            lhs,
                            exu1[b][:],
                        ).then_inc(sPE)

            @block.sync
            def _(sp):
                sp.dma_start(out=b16[:], in_=b16_d[:]).then_inc(sB16, 16)
                for b in range(BS):
                    sp.wait_ge(sDVE, 23 + b)  # dens fixup done
                    sp.wait_ge(sACT, 7 + 2 * b)  # field tanh done
                    sp.dma_start(out=out_d[b], in_=obuf[b][:]).then_inc(sOUT, 16)
                sp.wait_ge(sOUT, 32)

    return nc


def kernel(
    bezier_points,
    W1,
    b1,
    W2,
    b2,
    W3,
    b3,
    Wf,
    bf,
    kde_bandwidth,
    density_threshold,
    trace=False,
):
    global LAST_RESULT
    f32, f16 = np.float32, np.float16
    pts = np.asarray(bezier_points, f32).reshape(B, N, 2)
    W1, b1 = np.asarray(W1, f32), np.asarray(b1, f32)
    W2, b2 = np.asarray(W2, f32), np.asarray(b2, f32)
    W3, b3 = np.asarray(W3, f32), np.asarray(b3, f32)
    Wf, bf = np.asarray(Wf, f32), np.asarray(bf, f32)

    bw = max(float(np.float32(kde_bandwidth)), 1e-5)
    thr = float(np.float32(density_threshold))
    neg_inv2bw2 = -1.0 / (2.0 * bw * bw)
    c0 = math.exp(-1e-8 / (2.0 * bw * bw))
    s = 1.0 / (N * bw * math.sqrt(2.0 * math.pi))
    nc = _build(neg_inv2bw2, c0, 0.5 * s, -0.5 * thr)

    # host-side input marshalling (grid constants + weight transposes)
    blob32 = np.zeros((128, NC32), f32)
    blob32[:, C_GX : C_GX + W] = np.linspace(-1.0, 1.0, W, dtype=f32)
    blob32[:, C_GY : C_GY + H] = np.linspace(-1.0, 1.0, H, dtype=f32)
    blob32[:, C_BF : C_BF + 2] = bf
    blob32[:, C_B3] = b3
    blob16 = np.zeros((128, NC16), f16)
    blob16[:3, C_W1 : C_W1 + 64] = np.vstack([W1.T, b1[None, :]])
    blob16[:65, C_W2 : C_W2 + HID] = np.vstack([W2.T, b2[None, :]])
    blob16[:, C_W3 : C_W3 + HID] = W3.T
    blob16[:, C_WF : C_WF + 2] = Wf.T

    in_maps = []
    for i in range(NCORES):
        sh = pts[i * BS : (i + 1) * BS]  # [BS, N, 2]
        c32 = blob32.copy()
        for b in range(BS):
            c32[:, C_PC + 2 * b] = -sh[b, :, 0]
            c32[:, C_PC + 2 * b + 1] = -sh[b, :, 1]
        c16 = blob16.copy()
        c16[0, C_X0 : C_X0 + BS * N] = sh[..., 0].reshape(-1)
        c16[1, C_X0 : C_X0 + BS * N] = sh[..., 1].reshape(-1)
        c16[2, C_X0 : C_X0 + BS * N] = 1.0
        in_maps.append({"b32": c32, "b16": c16})

    res = run_bass_kernel_spmd(nc, in_maps, list(range(NCORES)), trace=trace)
    LAST_RESULT = res

    density = np.empty((B, 1, H, W), f32)
    field = np.empty((B, 2, H, W), f32)
    for i in range(NCORES):
        scr = res.results[i]["out_o"]  # [BS, 128, 1536]
        maps = scr.reshape(BS, 128, 3, 2, W).transpose(2, 0, 3, 1, 4)
        # maps[m, b, ch, p, w] with h = ch*128 + p
        density[i * BS : (i + 1) * BS, 0] = maps[0].reshape(BS, H, W)
        field[i * BS : (i + 1) * BS, 0] = maps[1].reshape(BS, H, W)
        field[i * BS : (i + 1) * BS, 1] = maps[2].reshape(BS, H, W)
    return density, field
